# revision 4
# baseline (speedup 1.0000x reference)
"""DetectionLayer decode kernel v7 for Trainium2 (Bass/Tile), 8-core SPMD.

Same math/dtype scheme as v6 (bf16 compressed rows padded to 86 cols,
f32 scores for the exact threshold, int32-AND row masking, bf16 output
upcast on host). v7 replaces the uniform 24x82-row tiling with a
variable schedule over each partition's contiguous row slab:
small tiles at both ends (first store issues ~8us earlier; final store
drains faster) and 96-row tiles in steady state.

Device bytes/core: ~46.8 MB read + 43.3 MB write.
"""
import sys

sys.path.insert(0, "/opt/trn_rl_repo")

import numpy as np
import ml_dtypes

import concourse.bacc as bacc
import concourse.mybir as mybir
from concourse.bass_utils import run_bass_kernel_spmd
from concourse.tile import TileContext

N = 2_000_000
C = 85
C2 = 86            # padded row length (43 int32 words)
W = C2 // 2        # int32 words per row
N_CORES = 8
P = 128            # SBUF partitions
M = 1968           # rows per partition per core (must be 1 mod 7 scaled: 7*S+R==N)
R = M * P          # 251,904 rows per core window
S = 249_728        # window stride; 7*S + R == N
# per-partition tile schedule (row counts): ramp in, steady 96s, ramp out
SCHED = [24, 48] + [96] * 19 + [48, 24]
THR = 0.5
BF16 = mybir.dt.bfloat16
I32 = mybir.dt.int32
F32 = mybir.dt.float32
NP_BF16 = ml_dtypes.bfloat16

assert sum(SCHED) == M
assert 7 * S + R == N and S % P == 0 and S <= R

_NC_CACHE = None


def _build_module():
    rows = R
    nc = bacc.Bacc("TRN2", target_bir_lowering=False, debug=False)
    inp = nc.dram_tensor("inputs", [rows, C2], BF16, kind="ExternalInput")
    anc = nc.dram_tensor("anchors", [rows, 4], BF16, kind="ExternalInput")
    sco = nc.dram_tensor("scores", [rows, 1], F32, kind="ExternalInput")
    out = nc.dram_tensor("out", [rows, C2], BF16, kind="ExternalOutput")

    # Partition p owns the contiguous row slab [p*M, (p+1)*M); a tile is any
    # row sub-range [a, b) of the slab (contiguous (b-a)*172B per partition).
    ivf = inp.ap().rearrange("(p m) c -> p (m c)", p=P)   # [128, M*C2]
    ovf = out.ap().rearrange("(p m) c -> p (m c)", p=P)
    av_all = anc.ap().rearrange("(p m) c -> p (m c)", p=P)  # [128, M*4]
    sv_all = sco.ap().rearrange("(p m) c -> p (m c)", p=P)  # [128, M]

    with TileContext(nc) as tc:
        with tc.tile_pool(name="anc", bufs=1) as apool, \
             tc.tile_pool(name="inp", bufs=5) as ipool, \
             tc.tile_pool(name="outp", bufs=4) as opool, \
             tc.tile_pool(name="amp", bufs=3) as mpool:
            anc_all = apool.tile([P, M * 4], BF16, tag="anc_all")
            sco_all = apool.tile([P, M], F32, tag="sco_all")
            mskf_all = apool.tile([P, M], F32, tag="mskf_all")
            mski_all = apool.tile([P, M], I32, tag="mski_all")
            # Scores first (they gate the masks and hence the first store),
            # on the otherwise-idle scalar HWDGE ring.
            nc.scalar.dma_start(out=sco_all[:], in_=sv_all)
            nc.scalar.dma_start(out=anc_all[:], in_=av_all)
            # Row masks: f32 1/0 (exact compare, feeds the am multiply) and
            # int32 0/~0 (passthrough AND; -1.0 -> int32 gives all-ones).
            nc.vector.tensor_single_scalar(
                mskf_all[:], sco_all[:], THR, mybir.AluOpType.is_gt
            )
            nc.vector.tensor_scalar_mul(mski_all[:], mskf_all[:], -1.0)
            a = 0
            for rows_t in SCHED:
                b = a + rows_t
                in_t = ipool.tile([P, rows_t * C2], BF16, tag="in")
                out_t = opool.tile([P, rows_t * C2], BF16, tag="out")
                am_t = mpool.tile([P, rows_t * 4], BF16, tag="am")

                nc.sync.dma_start(out=in_t[:], in_=ivf[:, a * C2:b * C2])

                ing = in_t[:].rearrange("p (g c) -> p g c", c=C2)
                outg = out_t[:].rearrange("p (g c) -> p g c", c=C2)
                inw = in_t[:].bitcast(I32).rearrange("p (g c) -> p g c", c=W)
                outw = out_t[:].bitcast(I32).rearrange("p (g c) -> p g c", c=W)
                ang = anc_all[:, a * 4:b * 4].rearrange("p (g c) -> p g c", c=4)
                amg = am_t[:].rearrange("p (g c) -> p g c", c=4)
                mbg = mskf_all[:, a:b].rearrange("p (g c) -> p g c", c=1)
                mig = mski_all[:, a:b].rearrange("p (g c) -> p g c", c=1)

                # out = mask & in (row-masked copy; cols 0..3 redone below)
                nc.vector.tensor_tensor(
                    outw, mig.broadcast_to([P, rows_t, W]), inw,
                    mybir.AluOpType.bitwise_and)
                # masked anchors: am = mask * anchors
                nc.vector.tensor_mul(amg, mbg.broadcast_to([P, rows_t, 4]), ang)
                # in[:, 2:4] = exp(in[:, 2:4]) in place on the scalar engine
                nc.scalar.activation(
                    ing[:, :, 2:4],
                    ing[:, :, 2:4],
                    mybir.ActivationFunctionType.Exp,
                )
                # out[:, 0:4] = [in_yx, exp(in_hw)] * [am_hw, am_hw]
                nc.vector.tensor_mul(
                    outg[:, :, 0:4].rearrange("p g (x y) -> p g x y", y=2),
                    ing[:, :, 0:4].rearrange("p g (x y) -> p g x y", y=2),
                    amg[:, :, 2:4].unsqueeze(2).broadcast_to([P, rows_t, 2, 2]),
                )
                # out[:, 0:2] += am_yx
                nc.vector.tensor_add(outg[:, :, 0:2], outg[:, :, 0:2], amg[:, :, 0:2])

                nc.gpsimd.dma_start(out=ovf[:, a * C2:b * C2], in_=out_t[:])
                a = b
    nc.compile()
    return nc


def _get_module():
    global _NC_CACHE
    if _NC_CACHE is None:
        _NC_CACHE = _build_module()
    return _NC_CACHE


def _run(inputs, anchors, **spmd_kwargs):
    inputs = np.asarray(inputs, dtype=np.float32)
    anchors = np.asarray(anchors, dtype=np.float32)
    assert inputs.shape == (N, C) and anchors.shape == (N, 4)

    scores = np.ascontiguousarray(inputs[:, 5:6])          # exact f32 scores
    inputs_bf = np.zeros((N, C2), dtype=NP_BF16)
    inputs_bf[:, :C] = inputs                              # cast-assign to bf16
    anchors_bf = anchors.astype(NP_BF16)

    nc = _get_module()
    in_maps = [
        {
            "inputs": inputs_bf[i * S : i * S + R],
            "anchors": anchors_bf[i * S : i * S + R],
            "scores": scores[i * S : i * S + R],
        }
        for i in range(N_CORES)
    ]
    res = run_bass_kernel_spmd(nc, in_maps, core_ids=list(range(N_CORES)), **spmd_kwargs)

    out = np.empty((N, C), dtype=np.float32)
    for i in range(N_CORES - 1):
        out[i * S : (i + 1) * S] = res.results[i]["out"][:S, :C]
    out[(N_CORES - 1) * S :] = res.results[N_CORES - 1]["out"][:, :C]
    return out, res


def kernel(inputs, anchors):
    out, _ = _run(inputs, anchors)
    return out


if __name__ == "__main__":
    rng = np.random.default_rng(0)
    x = rng.random((N, C), dtype=np.float32)
    a = rng.random((N, 4), dtype=np.float32)
    y = kernel(x, a)
    print("ran ok", y.shape, y.dtype)
